# revision 21
# baseline (speedup 1.0000x reference)
"""GQA causal attention (WINDOW==S so the sliding window is plain causal)
on 8 TRN2 NeuronCores.

Sharding: tensor-parallel over heads. Each core owns 4 contiguous Q heads
(= one KV-head group), computes its slice of Q/K/V projections, RoPE,
causal attention, and its partial contribution attn_c @ wo_c to the output;
the host sums the 8 partial outputs (bf16 partials, fp32 accumulate).

Key device-side choices (v2):
 - x pre-transposed on host to bf16 tiles [tb, f, t] so the contraction
   dim (features) lands on SBUF partitions with no on-device transpose.
 - Q/K RoPE'd tiles are transposed to [d, tok] via XBAR DMA transposes
   (dma_start_transpose) - zero tensor-engine transposes.
 - Attention output is computed TRANSPOSED: av[d,q] += V_i^T @ P_i with
   512-wide moving streams (pt tiles), so PE weight loads stay hidden and
   the result lands directly in the [d, tok] layout the wo matmul needs.
 - Softmax: exp((K'^T Q) - 7) on the scalar engine in fp16 (K pre-scaled
   by 1/sqrt(d) on host, -7 bias keeps fp16 in range; normalization
   cancels the shift). Denominator = ones-matmul over a vector-accumulated
   fp16 tile sum, reciprocal, then broadcast back to [128,q] via a 1-row
   matmul; one vector multiply normalizes av into AT.
 - PSUM tags: st(2)+qkv(2)+av(2)+bc(1)+den(1) = 8 banks. QT/KT/V/AT are
   double-buffered per batch so the Tile scheduler can overlap
   attention(b0) with QKV(b1) and attention(b1) with wo(b0), hiding the
   scalar-engine exp behind tensor work.
"""

import numpy as np

B, S, DIM = 2, 2048, 4096
NH, NKV, HD = 32, 8, 128
SCALE = HD ** -0.5
NCORES = 8
QH = NH // NCORES          # 4 q heads per core (one kv head)
TOK = B * S                # 4096 flattened tokens
TB = TOK // 128            # 32 token blocks
SB = S // 128              # 16 token blocks per batch
FB = DIM // 128            # 32 feature blocks
NEG = -1e9
EBIAS = -7.0               # exp(s - 7): keeps fp16 pt tiles in range

_cache = {}


def _build():
    import concourse.bass as bass
    import concourse.mybir as mybir
    import concourse.tile as tile
    from concourse import bacc
    from concourse.masks import make_identity

    dt = mybir.dt
    nc = bacc.Bacc("TRN2", target_bir_lowering=False, debug=False,
                   num_devices=NCORES)

    xT = nc.dram_tensor("xT", [TB, 128, FB * 128], dt.bfloat16,
                        kind="ExternalInput").ap()
    wqkv = nc.dram_tensor("wqkv", [FB, 128, 768], dt.bfloat16,
                          kind="ExternalInput").ap()
    wo4 = nc.dram_tensor("wo4", [QH, 128, DIM], dt.bfloat16,
                         kind="ExternalInput").ap()
    cos4 = nc.dram_tensor("cos4", [SB, 128, 256], dt.float32,
                          kind="ExternalInput").ap()
    sin4 = nc.dram_tensor("sin4", [SB, 128, 256], dt.float32,
                          kind="ExternalInput").ap()
    diag = nc.dram_tensor("diag", [128, 128], dt.float32,
                          kind="ExternalInput").ap()
    # chunk-major so each 128x512 store is one contiguous 128KB DMA
    out = nc.dram_tensor("out", [DIM // 512, TOK, 512], dt.bfloat16,
                         kind="ExternalOutput").ap()

    EXP = mybir.ActivationFunctionType.Exp
    COPY = mybir.ActivationFunctionType.Copy

    with tile.TileContext(nc) as tc:
        with (
            tc.tile_pool(name="const", bufs=1) as constp,
            tc.tile_pool(name="wqkvp", bufs=1) as wqkvp,
            tc.tile_pool(name="wop", bufs=2) as wop,
            tc.tile_pool(name="xtp", bufs=3) as xtp,
            tc.tile_pool(name="csp", bufs=3) as csp,
            tc.tile_pool(name="qkvo", bufs=2) as qkvo,      # QT/KT/V/AT (2 batches)
            tc.tile_pool(name="ropep", bufs=2) as ropep,
            tc.tile_pool(name="ptp", bufs=18) as ptp,
            tc.tile_pool(name="accp", bufs=2) as accp,
            tc.tile_pool(name="ocp", bufs=3) as ocp,
            tc.tile_pool(name="ps_st", bufs=2, space="PSUM") as ps_st,
            tc.tile_pool(name="ps_qk", bufs=2, space="PSUM") as ps_qk,
            tc.tile_pool(name="ps_av", bufs=2, space="PSUM") as ps_av,
            tc.tile_pool(name="ps_tp", bufs=1, space="PSUM") as ps_tp,
            tc.tile_pool(name="ps_dn", bufs=1, space="PSUM") as ps_dn,
        ):
            ident = constp.tile([128, 128], dt.bfloat16, tag="ident", name="ident")
            make_identity(nc, ident[:])
            dmask = constp.tile([128, 128], dt.float32, tag="dmask", name="dmask")
            nc.sync.dma_start(dmask[:], diag[:])
            cbias = constp.tile([128, 1], dt.float32, tag="cbias", name="cbias")
            nc.vector.memset(cbias[:], EBIAS)
            ones_k = constp.tile([128, 1], dt.float16, tag="ones_k", name="ones_k")
            nc.vector.memset(ones_k[:], 1.0)
            ones_1 = constp.tile([1, 128], dt.float16, tag="ones_1", name="ones_1")
            nc.vector.memset(ones_1[:], 1.0)

            # prefetch the first token-block inputs ahead of the bulk weight
            # load so the first projection chain starts as early as possible
            pre = {}

            def prefetch_block(sb, with_cs=True):
                xt = xtp.tile([128, FB, 128], dt.bfloat16, tag="xt",
                              name=f"xtp{sb}")
                nc.sync.dma_start(xt[:].rearrange("f fb t -> f (fb t)"), xT[sb])
                if with_cs:
                    cst = csp.tile([128, 256], dt.float32, tag="cos",
                                   name=f"cosp{sb}")
                    snt = csp.tile([128, 256], dt.float32, tag="sin",
                                   name=f"sinp{sb}")
                    nc.sync.dma_start(cst[:], cos4[sb])
                    nc.sync.dma_start(snt[:], sin4[sb])
                else:
                    cst = snt = None
                pre[sb] = (xt, cst, snt)

            wqkv_t = [wqkvp.tile([128, 768], dt.bfloat16, tag=f"wqkv{fb}",
                                 name=f"wqkv{fb}") for fb in range(FB)]
            # xt0 first at full bandwidth, first wqkv tiles right behind it
            # on the sync queue; the bulk of wqkv rides the scalar hwdge
            # queue in parallel
            xt0 = xtp.tile([128, FB, 128], dt.bfloat16, tag="xt", name="xtp0")
            nc.sync.dma_start(xt0[:].rearrange("f fb t -> f (fb t)"), xT[0])
            nc.sync.dma_start(wqkv_t[0][:], wqkv[0])
            nc.sync.dma_start(wqkv_t[1][:], wqkv[1])
            cst0 = csp.tile([128, 256], dt.float32, tag="cos", name="cosp0")
            snt0 = csp.tile([128, 256], dt.float32, tag="sin", name="sinp0")
            nc.sync.dma_start(cst0[:], cos4[0])
            nc.sync.dma_start(snt0[:], sin4[0])
            pre[0] = (xt0, cst0, snt0)
            prefetch_block(1)
            prefetch_block(2, with_cs=False)
            for fb in range(2, FB):
                nc.scalar.dma_start(wqkv_t[fb][:], wqkv[fb])

            # per-batch double-buffered activation tiles
            def alloc_batch_tiles():
                QT = [qkvo.tile([128, S], dt.bfloat16, tag=f"qt{h}", name=f"qt{h}")
                      for h in range(QH)]
                KT = qkvo.tile([128, S], dt.bfloat16, tag="kt", name="kt")
                V = [qkvo.tile([128, HD], dt.float16, tag=f"v{i}", name=f"v{i}")
                     for i in range(SB)]
                AT = [qkvo.tile([128, S], dt.bfloat16, tag=f"at{h}", name=f"at{h}")
                      for h in range(QH)]
                return QT, KT, V, AT

            def qkv_block(b, sb, bt, pools):
                """Project one 128-token block, rope, transpose into QT/KT/V."""
                QT, KT, V, _ = bt
                poolA, poolB = pools
                tb = b * SB + sb
                if b == 0 and sb in pre:
                    xt, cst, snt = pre.pop(sb)
                else:
                    xt = xtp.tile([128, FB, 128], dt.bfloat16, tag="xt", name="xt")
                    nc.sync.dma_start(xt[:].rearrange("f fb t -> f (fb t)"),
                                      xT[tb])
                    cst = snt = None
                if cst is None:
                    cst = csp.tile([128, 256], dt.float32, tag="cos", name="cos")
                    snt = csp.tile([128, 256], dt.float32, tag="sin", name="sin")
                    nc.sync.dma_start(cst[:], cos4[sb])
                    nc.sync.dma_start(snt[:], sin4[sb])

                psA = poolA.tile([128, 512], dt.float32, tag=poolA.name, name="psA")
                psB = poolB.tile([128, 512], dt.float32, tag=poolB.name, name="psB")
                for fb in range(FB):
                    nc.tensor.matmul(psA[:], xt[:, fb, :],
                                     wqkv_t[fb][:, 0:512],
                                     start=(fb == 0), stop=(fb == FB - 1))
                    nc.tensor.matmul(psB[:, 0:256], xt[:, fb, :],
                                     wqkv_t[fb][:, 512:768],
                                     start=(fb == 0), stop=(fb == FB - 1))

                # RoPE on Q: [tok, 512] interleaved pairs
                rq = ropep.tile([128, 512], dt.bfloat16, tag="rq", name="rq")
                qa = psA[:].rearrange("p (i two) -> p two i", two=2)
                ra = rq[:].rearrange("p (i two) -> p two i", two=2)
                t1 = ropep.tile([128, 256], dt.float32, tag="t1", name="t1", bufs=1)
                t2 = ropep.tile([128, 256], dt.float32, tag="t2", name="t2", bufs=1)
                t3 = ropep.tile([128, 256], dt.float32, tag="t3", name="t3", bufs=1)
                t4 = ropep.tile([128, 256], dt.float32, tag="t4", name="t4", bufs=1)
                nc.vector.tensor_mul(t1[:], qa[:, 0, :], cst[:])
                nc.vector.tensor_mul(t2[:], qa[:, 1, :], snt[:])
                nc.vector.tensor_mul(t3[:], qa[:, 0, :], snt[:])
                nc.vector.tensor_mul(t4[:], qa[:, 1, :], cst[:])
                nc.vector.tensor_sub(ra[:, 0, :], t1[:], t2[:])
                nc.vector.tensor_add(ra[:, 1, :], t3[:], t4[:])

                # RoPE on K: [tok, 128] (wk pre-scaled by 1/sqrt(d) on host)
                rk = ropep.tile([128, 128], dt.bfloat16, tag="rk", name="rk")
                ka = psB[:, 0:128].rearrange("p (i two) -> p two i", two=2)
                rka = rk[:].rearrange("p (i two) -> p two i", two=2)
                t5 = ropep.tile([128, 64], dt.float32, tag="t5", name="t5", bufs=1)
                t6 = ropep.tile([128, 64], dt.float32, tag="t6", name="t6", bufs=1)
                # V copy first so psB's slot frees as early as possible
                nc.vector.tensor_copy(V[sb][:], psB[:, 128:256])
                t7 = ropep.tile([128, 64], dt.float32, tag="t5x", name="t7", bufs=1)
                t8 = ropep.tile([128, 64], dt.float32, tag="t6x", name="t8", bufs=1)
                nc.vector.tensor_mul(t5[:], ka[:, 0, :], cst[:, 0:64])
                nc.vector.tensor_mul(t6[:], ka[:, 1, :], snt[:, 0:64])
                nc.vector.tensor_mul(t7[:], ka[:, 0, :], snt[:, 0:64])
                nc.vector.tensor_mul(t8[:], ka[:, 1, :], cst[:, 0:64])
                nc.vector.tensor_sub(rka[:, 0, :], t5[:], t6[:])
                nc.vector.tensor_add(rka[:, 1, :], t7[:], t8[:])

                # PE transposes into [d, tok] layout; copies on scalar engine
                for h in range(QH):
                    tp = ps_tp.tile([128, 128], dt.bfloat16, tag="ps_tp",
                                    name="tpq")
                    nc.tensor.transpose(tp[:], rq[:, h * 128:(h + 1) * 128],
                                        ident[:])
                    nc.scalar.activation(QT[h][:, sb * 128:(sb + 1) * 128],
                                         tp[:], COPY)
                tpk = ps_tp.tile([128, 128], dt.bfloat16, tag="ps_tp", name="tpk")
                nc.tensor.transpose(tpk[:], rk[:], ident[:])
                nc.scalar.activation(KT[:, sb * 128:(sb + 1) * 128], tpk[:], COPY)

            def attn_head(b, h, bt, filler=None):
                """Causal attention for one head: scores -> exp -> avT -> norm."""
                QT, KT, V, AT = bt
                for j in range(4):
                    ptiles = []
                    for i in range(4 * j + 4):
                        off = max(0, i - 4 * j) * 128
                        st = ps_st.tile([128, 512], dt.float32, tag="ps_st",
                                        name="st")
                        nc.tensor.matmul(
                            st[:, off:512],
                            KT[:, i * 128:(i + 1) * 128],
                            QT[h][:, j * 512 + off:(j + 1) * 512],
                            start=True, stop=True)
                        if i >= 4 * j:
                            nc.vector.tensor_add(st[:, off:off + 128],
                                                 st[:, off:off + 128],
                                                 dmask[:])
                        pt = ptp.tile([128, 512], dt.float16, tag="pt", name="pt")
                        nc.scalar.activation(pt[:, off:512], st[:, off:512],
                                             EXP, bias=cbias[:], scale=1.0)
                        ptiles.append((pt, off))

                    # avT[d, q] accumulated over key blocks; 512-wide streams
                    av = ps_av.tile([128, 512], dt.float32, tag="ps_av", name="av")
                    n = len(ptiles)
                    for i, (pt, off) in enumerate(ptiles):
                        nc.tensor.matmul(av[:, off:512], V[i][:], pt[:, off:512],
                                         start=(i == 0), stop=(i == n - 1))

                    # denominator: acc16 = sum_i pt_i (fp16, DVE 4x mode)
                    acc16 = accp.tile([128, 512], dt.float16, tag="acc16",
                                      name="acc16")
                    nc.vector.tensor_copy(acc16[:], ptiles[0][0][:])
                    for pt, off in ptiles[1:]:
                        nc.vector.tensor_add(acc16[:, off:512],
                                             acc16[:, off:512], pt[:, off:512])
                    den = ps_dn.tile([1, 512], dt.float32, tag="ps_dn", name="den")
                    nc.tensor.matmul(den[:], ones_k[:], acc16[:],
                                     start=True, stop=True)
                    recf = accp.tile([1, 512], dt.float32, tag="recf", name="recf",
                                     bufs=1)
                    nc.vector.reciprocal_approx_fast(recf[:], den[:])
                    rec = accp.tile([1, 512], dt.float16, tag="rec", name="rec",
                                    bufs=1)
                    nc.vector.tensor_copy(rec[:], recf[:])
                    bc = ps_dn.tile([128, 512], dt.float32, tag="ps_dn", name="bc")
                    nc.tensor.matmul(bc[:], ones_1[:], rec[:],
                                     start=True, stop=True)
                    bcs = accp.tile([128, 512], dt.float16, tag="bcs", name="bcs")
                    nc.vector.tensor_copy(bcs[:], bc[:])
                    nc.vector.tensor_mul(AT[h][:, j * 512:(j + 1) * 512],
                                         av[:], bcs[:])
                    if filler is not None:
                        filler()

            def wo_load(ch):
                wo_t = []
                for h in range(QH):
                    w = wop.tile([128, 512], dt.bfloat16, tag=f"wo{h}",
                                 name=f"wo{h}")
                    nc.sync.dma_start(w[:], wo4[h, :, ch * 512:(ch + 1) * 512])
                    wo_t.append(w)
                return wo_t

            def make_wo_filler(b, bt, ps_pools, oc_engines):
                """Atom-level generator over all 8 wo chunks x 16 token
                blocks of one batch. Output staged in [128, 2, 512] oc tiles
                covering two token blocks -> one 256-row store each, queues
                alternating between the gpsimd software DGE and sync."""
                AT = bt[3]
                state = {"i": 0, "wt": {0: wo_load(0)}, "oc": None}

                def emit(n):
                    for _ in range(n):
                        i = state["i"]
                        if i >= 8 * SB:
                            return
                        ch, sb = divmod(i, SB)
                        state["i"] += 1
                        if sb == 0 and ch + 1 < 8:
                            state["wt"][ch + 1] = wo_load(ch + 1)
                        wo_t = state["wt"][ch]
                        pool = ps_pools[i % len(ps_pools)]
                        ps = pool.tile([128, 512], dt.float32, tag=pool.name,
                                       name="ps")
                        for h in range(QH):
                            nc.tensor.matmul(
                                ps[:], AT[h][:, sb * 128:(sb + 1) * 128],
                                wo_t[h][:], start=(h == 0), stop=(h == QH - 1))
                        if sb % 2 == 0:
                            state["oc"] = ocp.tile([128, 2, 512], dt.bfloat16,
                                                   tag="oc", name="oc")
                        oc = state["oc"]
                        eng = oc_engines[sb % len(oc_engines)]
                        if eng == "scalar":
                            nc.scalar.activation(oc[:, sb % 2, :], ps[:], COPY)
                        else:
                            nc.vector.tensor_copy(oc[:, sb % 2, :], ps[:])
                        if sb % 2 == 1:
                            dge = nc.gpsimd if (sb // 2) % 2 == 0 else nc.sync
                            t0 = b * S + (sb - 1) * 128
                            dge.dma_start(
                                out[ch, t0:t0 + 256, :].rearrange(
                                    "(two p) c -> p two c", two=2),
                                oc[:])
                        if sb == SB - 1:
                            state["wt"].pop(ch)

                return emit

            # ---- phase A: QKV(b0), 4-deep psum pipeline across st/qkv tags
            bt0 = alloc_batch_tiles()
            for sb in range(SB):
                pools = (ps_st, ps_qk) if sb % 2 == 0 else (ps_qk, ps_st)
                qkv_block(0, sb, bt0, pools)

            # ---- phase B (x) D: attention(b0) overlapped with QKV(b1),
            # one projection block emitted after each attention j-block
            bt1 = alloc_batch_tiles()
            dstate = {"sb": 0}

            def qkv_filler():
                if dstate["sb"] < SB:
                    qkv_block(1, dstate["sb"], bt1, (ps_qk, ps_qk))
                    dstate["sb"] += 1

            for h in range(QH):
                attn_head(0, h, bt0, filler=qkv_filler)
            while dstate["sb"] < SB:
                qkv_filler()

            # ---- phase C (x) E: wo(b0) overlapped with attention(b1),
            # eight wo atoms emitted after each attention j-block
            wo0 = make_wo_filler(0, bt0, [ps_qk], ["scalar", "vector"])
            for h in range(QH):
                attn_head(1, h, bt1, filler=lambda: wo0(8))
            wo0(8 * SB)   # drain any remainder

            # ---- phase F: wo(b1), deep psum rotation across all free tags
            wo1 = make_wo_filler(1, bt1, [ps_qk, ps_st, ps_av],
                                 ["scalar", "vector"])
            wo1(8 * SB)

    nc.compile()
    return nc


def _prep_host(inputs):
    import ml_dtypes
    bf16 = ml_dtypes.bfloat16

    x = np.asarray(inputs["x"], np.float32)
    wq = np.asarray(inputs["wq"], np.float32)
    wk = np.asarray(inputs["wk"], np.float32)
    wv = np.asarray(inputs["wv"], np.float32)
    wo = np.asarray(inputs["wo"], np.float32)
    cos = np.asarray(inputs["freqs_cos"], np.float32)
    sin = np.asarray(inputs["freqs_sin"], np.float32)

    x2 = x.reshape(TOK, DIM)
    xT5 = np.ascontiguousarray(
        x2.reshape(TB, 128, FB, 128).transpose(0, 3, 2, 1)
        .reshape(TB, 128, FB * 128)).astype(bf16)
    cos4 = np.ascontiguousarray(
        np.tile(cos, (1, QH)).reshape(SB, 128, 256)).astype(np.float32)
    sin4 = np.ascontiguousarray(
        np.tile(sin, (1, QH)).reshape(SB, 128, 256)).astype(np.float32)
    k_i = np.arange(128)[:, None]
    q_i = np.arange(128)[None, :]
    dmask = np.where(k_i <= q_i, 0.0, NEG).astype(np.float32)

    in_maps = []
    for c in range(NCORES):
        wq_c = wq[:, c * QH * HD:(c + 1) * QH * HD]
        wk_c = wk[:, c * HD:(c + 1) * HD] * SCALE   # fold softmax scale into K
        wv_c = wv[:, c * HD:(c + 1) * HD]
        wqkv_c = np.ascontiguousarray(
            np.concatenate([wq_c, wk_c, wv_c], axis=1)
            .reshape(FB, 128, 768)).astype(bf16)
        wo_c = np.ascontiguousarray(
            wo[c * QH * HD:(c + 1) * QH * HD, :]
            .reshape(QH, HD, DIM)).astype(bf16)
        in_maps.append({
            "xT": xT5, "wqkv": wqkv_c, "wo4": wo_c,
            "cos4": cos4, "sin4": sin4, "diag": dmask,
        })
    return in_maps


def run_on_device(inputs, trace=False, tmpdir=None):
    """Compile (cached) + run; returns (full_output, BassKernelResults)."""
    import sys
    if "/opt/trn_rl_repo" not in sys.path:
        sys.path.insert(0, "/opt/trn_rl_repo")
    from concourse.bass_utils import run_bass_kernel_spmd

    if "nc" not in _cache:
        _cache["nc"] = _build()
    nc = _cache["nc"]
    in_maps = _prep_host(inputs)
    res = run_bass_kernel_spmd(nc, in_maps, core_ids=list(range(NCORES)),
                               trace=trace, tmpdir=tmpdir)
    acc = np.zeros((DIM // 512, TOK, 512), np.float32)
    for c in range(NCORES):
        acc += np.asarray(res.results[c]["out"], np.float32)
    full = np.ascontiguousarray(acc.transpose(1, 0, 2)).reshape(TOK, DIM)
    return full.reshape(B, S, DIM), res


def kernel(**inputs):
    out, _ = run_on_device(inputs, trace=False)
    return out


# revision 22
# speedup vs baseline: 1.1831x; 1.1831x over previous
"""GQA causal attention (WINDOW==S so the sliding window is plain causal)
on 8 TRN2 NeuronCores.

Sharding: tensor-parallel over heads. Each core owns 4 contiguous Q heads
(= one KV-head group), computes its slice of Q/K/V projections, RoPE,
causal attention, and its partial contribution attn_c @ wo_c to the output;
the host sums the 8 partial outputs (bf16 partials, fp32 accumulate).

Key device-side choices (v2):
 - x pre-transposed on host to bf16 tiles [tb, f, t] so the contraction
   dim (features) lands on SBUF partitions with no on-device transpose.
 - Q/K RoPE'd tiles are transposed to [d, tok] via XBAR DMA transposes
   (dma_start_transpose) - zero tensor-engine transposes.
 - Attention output is computed TRANSPOSED: av[d,q] += V_i^T @ P_i with
   512-wide moving streams (pt tiles), so PE weight loads stay hidden and
   the result lands directly in the [d, tok] layout the wo matmul needs.
 - Softmax: exp((K'^T Q) - 7) on the scalar engine in fp16 (K pre-scaled
   by 1/sqrt(d) on host, -7 bias keeps fp16 in range; normalization
   cancels the shift). Denominator = ones-matmul over a vector-accumulated
   fp16 tile sum, reciprocal, then broadcast back to [128,q] via a 1-row
   matmul; one vector multiply normalizes av into AT.
 - PSUM tags: st(2)+qkv(2)+av(2)+bc(1)+den(1) = 8 banks. QT/KT/V/AT are
   double-buffered per batch so the Tile scheduler can overlap
   attention(b0) with QKV(b1) and attention(b1) with wo(b0), hiding the
   scalar-engine exp behind tensor work.
"""

import numpy as np

B, S, DIM = 2, 2048, 4096
NH, NKV, HD = 32, 8, 128
SCALE = HD ** -0.5
NCORES = 8
QH = NH // NCORES          # 4 q heads per core (one kv head)
TOK = B * S                # 4096 flattened tokens
TB = TOK // 128            # 32 token blocks
SB = S // 128              # 16 token blocks per batch
FB = DIM // 128            # 32 feature blocks
NEG = -1e9
EBIAS = -7.0               # exp(s - 7): keeps fp16 pt tiles in range

_cache = {}


def _build():
    import concourse.bass as bass
    import concourse.mybir as mybir
    import concourse.tile as tile
    from concourse import bacc
    from concourse.masks import make_identity

    dt = mybir.dt
    nc = bacc.Bacc("TRN2", target_bir_lowering=False, debug=False,
                   num_devices=NCORES)

    xT = nc.dram_tensor("xT", [TB, 128, FB * 128], dt.bfloat16,
                        kind="ExternalInput").ap()
    wqkv = nc.dram_tensor("wqkv", [FB, 128, 768], dt.bfloat16,
                          kind="ExternalInput").ap()
    wo4 = nc.dram_tensor("wo4", [QH, 128, DIM], dt.bfloat16,
                         kind="ExternalInput").ap()
    cos4 = nc.dram_tensor("cos4", [SB, 128, 256], dt.float32,
                          kind="ExternalInput").ap()
    sin4 = nc.dram_tensor("sin4", [SB, 128, 256], dt.float32,
                          kind="ExternalInput").ap()
    diag = nc.dram_tensor("diag", [128, 128], dt.float32,
                          kind="ExternalInput").ap()
    # chunk-major so each 128x512 store is one contiguous 128KB DMA
    out = nc.dram_tensor("out", [DIM // 512, TOK, 512], dt.bfloat16,
                         kind="ExternalOutput").ap()

    EXP = mybir.ActivationFunctionType.Exp
    COPY = mybir.ActivationFunctionType.Copy

    with tile.TileContext(nc) as tc:
        with (
            tc.tile_pool(name="const", bufs=1) as constp,
            tc.tile_pool(name="wqkvp", bufs=1) as wqkvp,
            tc.tile_pool(name="wop", bufs=2) as wop,
            tc.tile_pool(name="xtp", bufs=3) as xtp,
            tc.tile_pool(name="csp", bufs=3) as csp,
            tc.tile_pool(name="qkvo", bufs=2) as qkvo,      # QT/KT/V/AT (2 batches)
            tc.tile_pool(name="ropep", bufs=2) as ropep,
            tc.tile_pool(name="ptp", bufs=18) as ptp,
            tc.tile_pool(name="accp", bufs=2) as accp,
            tc.tile_pool(name="ocp", bufs=3) as ocp,
            tc.tile_pool(name="ps_st", bufs=2, space="PSUM") as ps_st,
            tc.tile_pool(name="ps_qk", bufs=2, space="PSUM") as ps_qk,
            tc.tile_pool(name="ps_av", bufs=2, space="PSUM") as ps_av,
            tc.tile_pool(name="ps_tp", bufs=1, space="PSUM") as ps_tp,
            tc.tile_pool(name="ps_dn", bufs=1, space="PSUM") as ps_dn,
        ):
            ident = constp.tile([128, 128], dt.bfloat16, tag="ident", name="ident")
            make_identity(nc, ident[:])
            dmask = constp.tile([128, 128], dt.float32, tag="dmask", name="dmask")
            nc.sync.dma_start(dmask[:], diag[:])
            cbias = constp.tile([128, 1], dt.float32, tag="cbias", name="cbias")
            nc.vector.memset(cbias[:], EBIAS)
            ones_k = constp.tile([128, 1], dt.float16, tag="ones_k", name="ones_k")
            nc.vector.memset(ones_k[:], 1.0)
            ones_1 = constp.tile([1, 128], dt.float16, tag="ones_1", name="ones_1")
            nc.vector.memset(ones_1[:], 1.0)

            # prefetch the first token-block inputs ahead of the bulk weight
            # load so the first projection chain starts as early as possible
            pre = {}

            def prefetch_block(sb, with_cs=True):
                xt = xtp.tile([128, FB, 128], dt.bfloat16, tag="xt",
                              name=f"xtp{sb}")
                nc.sync.dma_start(xt[:].rearrange("f fb t -> f (fb t)"), xT[sb])
                if with_cs:
                    cst = csp.tile([128, 256], dt.float32, tag="cos",
                                   name=f"cosp{sb}")
                    snt = csp.tile([128, 256], dt.float32, tag="sin",
                                   name=f"sinp{sb}")
                    nc.sync.dma_start(cst[:], cos4[sb])
                    nc.sync.dma_start(snt[:], sin4[sb])
                else:
                    cst = snt = None
                pre[sb] = (xt, cst, snt)

            wqkv_t = [wqkvp.tile([128, 768], dt.bfloat16, tag=f"wqkv{fb}",
                                 name=f"wqkv{fb}") for fb in range(FB)]
            # xt0 first at full bandwidth, first wqkv tiles right behind it
            # on the sync queue; the bulk of wqkv rides the scalar hwdge
            # queue in parallel
            xt0 = xtp.tile([128, FB, 128], dt.bfloat16, tag="xt", name="xtp0")
            nc.sync.dma_start(xt0[:].rearrange("f fb t -> f (fb t)"), xT[0])
            nc.sync.dma_start(wqkv_t[0][:], wqkv[0])
            nc.sync.dma_start(wqkv_t[1][:], wqkv[1])
            cst0 = csp.tile([128, 256], dt.float32, tag="cos", name="cosp0")
            snt0 = csp.tile([128, 256], dt.float32, tag="sin", name="sinp0")
            nc.sync.dma_start(cst0[:], cos4[0])
            nc.sync.dma_start(snt0[:], sin4[0])
            pre[0] = (xt0, cst0, snt0)
            prefetch_block(1)
            prefetch_block(2, with_cs=False)
            for fb in range(2, FB):
                nc.scalar.dma_start(wqkv_t[fb][:], wqkv[fb])

            # per-batch double-buffered activation tiles
            def alloc_batch_tiles():
                QT = [qkvo.tile([128, S], dt.bfloat16, tag=f"qt{h}", name=f"qt{h}")
                      for h in range(QH)]
                KT = qkvo.tile([128, S], dt.bfloat16, tag="kt", name="kt")
                V = [qkvo.tile([128, HD], dt.float16, tag=f"v{i}", name=f"v{i}")
                     for i in range(SB)]
                AT = [qkvo.tile([128, S], dt.bfloat16, tag=f"at{h}", name=f"at{h}")
                      for h in range(QH)]
                return QT, KT, V, AT

            def qkv_block(b, sb, bt, pools):
                """Project one 128-token block, rope, transpose into QT/KT/V."""
                QT, KT, V, _ = bt
                poolA, poolB = pools
                tb = b * SB + sb
                if b == 0 and sb in pre:
                    xt, cst, snt = pre.pop(sb)
                else:
                    xt = xtp.tile([128, FB, 128], dt.bfloat16, tag="xt", name="xt")
                    nc.sync.dma_start(xt[:].rearrange("f fb t -> f (fb t)"),
                                      xT[tb])
                    cst = snt = None
                if cst is None:
                    cst = csp.tile([128, 256], dt.float32, tag="cos", name="cos")
                    snt = csp.tile([128, 256], dt.float32, tag="sin", name="sin")
                    nc.sync.dma_start(cst[:], cos4[sb])
                    nc.sync.dma_start(snt[:], sin4[sb])

                psA = poolA.tile([128, 512], dt.float32, tag=poolA.name, name="psA")
                psB = poolB.tile([128, 512], dt.float32, tag=poolB.name, name="psB")
                for fb in range(FB):
                    nc.tensor.matmul(psA[:], xt[:, fb, :],
                                     wqkv_t[fb][:, 0:512],
                                     start=(fb == 0), stop=(fb == FB - 1))
                    nc.tensor.matmul(psB[:, 0:256], xt[:, fb, :],
                                     wqkv_t[fb][:, 512:768],
                                     start=(fb == 0), stop=(fb == FB - 1))

                # RoPE on Q: [tok, 512] interleaved pairs
                rq = ropep.tile([128, 512], dt.bfloat16, tag="rq", name="rq")
                qa = psA[:].rearrange("p (i two) -> p two i", two=2)
                ra = rq[:].rearrange("p (i two) -> p two i", two=2)
                t1 = ropep.tile([128, 256], dt.float32, tag="t1", name="t1", bufs=1)
                t2 = ropep.tile([128, 256], dt.float32, tag="t2", name="t2", bufs=1)
                t3 = ropep.tile([128, 256], dt.float32, tag="t3", name="t3", bufs=1)
                t4 = ropep.tile([128, 256], dt.float32, tag="t4", name="t4", bufs=1)
                nc.vector.tensor_mul(t1[:], qa[:, 0, :], cst[:])
                nc.vector.tensor_mul(t2[:], qa[:, 1, :], snt[:])
                nc.vector.tensor_mul(t3[:], qa[:, 0, :], snt[:])
                nc.vector.tensor_mul(t4[:], qa[:, 1, :], cst[:])
                nc.vector.tensor_sub(ra[:, 0, :], t1[:], t2[:])
                nc.vector.tensor_add(ra[:, 1, :], t3[:], t4[:])

                # RoPE on K: [tok, 128] (wk pre-scaled by 1/sqrt(d) on host)
                rk = ropep.tile([128, 128], dt.bfloat16, tag="rk", name="rk")
                ka = psB[:, 0:128].rearrange("p (i two) -> p two i", two=2)
                rka = rk[:].rearrange("p (i two) -> p two i", two=2)
                t5 = ropep.tile([128, 64], dt.float32, tag="t5", name="t5", bufs=1)
                t6 = ropep.tile([128, 64], dt.float32, tag="t6", name="t6", bufs=1)
                # V copy first so psB's slot frees as early as possible
                nc.vector.tensor_copy(V[sb][:], psB[:, 128:256])
                t7 = ropep.tile([128, 64], dt.float32, tag="t5x", name="t7", bufs=1)
                t8 = ropep.tile([128, 64], dt.float32, tag="t6x", name="t8", bufs=1)
                nc.vector.tensor_mul(t5[:], ka[:, 0, :], cst[:, 0:64])
                nc.vector.tensor_mul(t6[:], ka[:, 1, :], snt[:, 0:64])
                nc.vector.tensor_mul(t7[:], ka[:, 0, :], snt[:, 0:64])
                nc.vector.tensor_mul(t8[:], ka[:, 1, :], cst[:, 0:64])
                nc.vector.tensor_sub(rka[:, 0, :], t5[:], t6[:])
                nc.vector.tensor_add(rka[:, 1, :], t7[:], t8[:])

                # PE transposes into [d, tok] layout; copies on scalar engine
                for h in range(QH):
                    tp = ps_tp.tile([128, 128], dt.bfloat16, tag="ps_tp",
                                    name="tpq")
                    nc.tensor.transpose(tp[:], rq[:, h * 128:(h + 1) * 128],
                                        ident[:])
                    nc.scalar.activation(QT[h][:, sb * 128:(sb + 1) * 128],
                                         tp[:], COPY)
                tpk = ps_tp.tile([128, 128], dt.bfloat16, tag="ps_tp", name="tpk")
                nc.tensor.transpose(tpk[:], rk[:], ident[:])
                nc.scalar.activation(KT[:, sb * 128:(sb + 1) * 128], tpk[:], COPY)

            def attn_head(b, h, bt, filler=None):
                """Causal attention for one head: scores -> exp -> avT -> norm."""
                QT, KT, V, AT = bt
                for j in range(4):
                    ptiles = []
                    for i in range(4 * j + 4):
                        off = max(0, i - 4 * j) * 128
                        st = ps_st.tile([128, 512], dt.float32, tag="ps_st",
                                        name="st")
                        nc.tensor.matmul(
                            st[:, off:512],
                            KT[:, i * 128:(i + 1) * 128],
                            QT[h][:, j * 512 + off:(j + 1) * 512],
                            start=True, stop=True)
                        if i >= 4 * j:
                            nc.vector.tensor_add(st[:, off:off + 128],
                                                 st[:, off:off + 128],
                                                 dmask[:])
                        pt = ptp.tile([128, 512], dt.float16, tag="pt", name="pt")
                        nc.scalar.activation(pt[:, off:512], st[:, off:512],
                                             EXP, bias=cbias[:], scale=1.0)
                        ptiles.append((pt, off))

                    # avT[d, q] accumulated over key blocks; 512-wide streams
                    av = ps_av.tile([128, 512], dt.float32, tag="ps_av", name="av")
                    n = len(ptiles)
                    for i, (pt, off) in enumerate(ptiles):
                        nc.tensor.matmul(av[:, off:512], V[i][:], pt[:, off:512],
                                         start=(i == 0), stop=(i == n - 1))

                    # denominator: acc16 = sum_i pt_i (fp16, DVE 4x mode)
                    acc16 = accp.tile([128, 512], dt.float16, tag="acc16",
                                      name="acc16")
                    nc.vector.tensor_copy(acc16[:], ptiles[0][0][:])
                    for pt, off in ptiles[1:]:
                        nc.vector.tensor_add(acc16[:, off:512],
                                             acc16[:, off:512], pt[:, off:512])
                    den = ps_dn.tile([1, 512], dt.float32, tag="ps_dn", name="den")
                    nc.tensor.matmul(den[:], ones_k[:], acc16[:],
                                     start=True, stop=True)
                    recf = accp.tile([1, 512], dt.float32, tag="recf", name="recf",
                                     bufs=1)
                    nc.vector.reciprocal_approx_fast(recf[:], den[:])
                    rec = accp.tile([1, 512], dt.float16, tag="rec", name="rec",
                                    bufs=1)
                    nc.vector.tensor_copy(rec[:], recf[:])
                    bc = ps_dn.tile([128, 512], dt.float32, tag="ps_dn", name="bc")
                    nc.tensor.matmul(bc[:], ones_1[:], rec[:],
                                     start=True, stop=True)
                    bcs = accp.tile([128, 512], dt.float16, tag="bcs", name="bcs")
                    nc.vector.tensor_copy(bcs[:], bc[:])
                    nc.vector.tensor_mul(AT[h][:, j * 512:(j + 1) * 512],
                                         av[:], bcs[:])
                    if filler is not None:
                        filler()

            def wo_load(ch):
                wo_t = []
                for h in range(QH):
                    w = wop.tile([128, 512], dt.bfloat16, tag=f"wo{h}",
                                 name=f"wo{h}")
                    nc.sync.dma_start(w[:], wo4[h, :, ch * 512:(ch + 1) * 512])
                    wo_t.append(w)
                return wo_t

            def make_wo_filler(b, bt, ps_pools, oc_engines):
                """Atom-level generator over all 8 wo chunks x 16 token
                blocks of one batch. Output staged in [128, 2, 512] oc tiles
                covering two token blocks -> one 256-row store each, queues
                alternating between the gpsimd software DGE and sync."""
                AT = bt[3]
                state = {"i": 0, "wt": {0: wo_load(0)}, "oc": None}

                def emit(n):
                    for _ in range(n):
                        i = state["i"]
                        if i >= 8 * SB:
                            return
                        ch, sb = divmod(i, SB)
                        state["i"] += 1
                        if sb == 0 and ch + 1 < 8:
                            state["wt"][ch + 1] = wo_load(ch + 1)
                        wo_t = state["wt"][ch]
                        pool = ps_pools[i % len(ps_pools)]
                        ps = pool.tile([128, 512], dt.float32, tag=pool.name,
                                       name="ps")
                        for h in range(QH):
                            nc.tensor.matmul(
                                ps[:], AT[h][:, sb * 128:(sb + 1) * 128],
                                wo_t[h][:], start=(h == 0), stop=(h == QH - 1))
                        if sb % 2 == 0:
                            state["oc"] = ocp.tile([128, 2, 512], dt.bfloat16,
                                                   tag="oc", name="oc")
                        oc = state["oc"]
                        eng = oc_engines[sb % len(oc_engines)]
                        if eng == "scalar":
                            nc.scalar.activation(oc[:, sb % 2, :], ps[:], COPY)
                        else:
                            nc.vector.tensor_copy(oc[:, sb % 2, :], ps[:])
                        if sb % 2 == 1:
                            dge = nc.gpsimd if (sb // 2) % 2 == 0 else nc.sync
                            t0 = b * S + (sb - 1) * 128
                            dge.dma_start(
                                out[ch, t0:t0 + 256, :].rearrange(
                                    "(two p) c -> p two c", two=2),
                                oc[:])
                        if sb == SB - 1:
                            state["wt"].pop(ch)

                return emit

            # ---- phase A: QKV(b0), 4-deep psum pipeline across st/qkv tags
            bt0 = alloc_batch_tiles()
            for sb in range(SB):
                pools = (ps_st, ps_qk) if sb % 2 == 0 else (ps_qk, ps_st)
                qkv_block(0, sb, bt0, pools)

            # ---- phase B (x) D: attention(b0) overlapped with QKV(b1),
            # one projection block emitted after each attention j-block
            bt1 = alloc_batch_tiles()
            dstate = {"sb": 0}

            def qkv_filler():
                if dstate["sb"] < SB:
                    qkv_block(1, dstate["sb"], bt1, (ps_qk, ps_qk))
                    dstate["sb"] += 1

            for h in range(QH):
                attn_head(0, h, bt0)
                for _ in range(4):
                    qkv_filler()
            while dstate["sb"] < SB:
                qkv_filler()

            # ---- phase C (x) E: wo(b0) overlapped with attention(b1),
            # eight wo atoms emitted after each attention j-block
            wo0 = make_wo_filler(0, bt0, [ps_qk], ["scalar", "vector"])
            for h in range(QH):
                wo0(SB)
                attn_head(1, h, bt1)
                wo0(SB)
            wo0(8 * SB)   # drain any remainder

            # ---- phase F: wo(b1), deep psum rotation across all free tags
            wo1 = make_wo_filler(1, bt1, [ps_qk, ps_st, ps_av],
                                 ["scalar", "vector"])
            wo1(8 * SB)

    nc.compile()
    return nc


def _prep_host(inputs):
    import ml_dtypes
    bf16 = ml_dtypes.bfloat16

    x = np.asarray(inputs["x"], np.float32)
    wq = np.asarray(inputs["wq"], np.float32)
    wk = np.asarray(inputs["wk"], np.float32)
    wv = np.asarray(inputs["wv"], np.float32)
    wo = np.asarray(inputs["wo"], np.float32)
    cos = np.asarray(inputs["freqs_cos"], np.float32)
    sin = np.asarray(inputs["freqs_sin"], np.float32)

    x2 = x.reshape(TOK, DIM)
    xT5 = np.ascontiguousarray(
        x2.reshape(TB, 128, FB, 128).transpose(0, 3, 2, 1)
        .reshape(TB, 128, FB * 128)).astype(bf16)
    cos4 = np.ascontiguousarray(
        np.tile(cos, (1, QH)).reshape(SB, 128, 256)).astype(np.float32)
    sin4 = np.ascontiguousarray(
        np.tile(sin, (1, QH)).reshape(SB, 128, 256)).astype(np.float32)
    k_i = np.arange(128)[:, None]
    q_i = np.arange(128)[None, :]
    dmask = np.where(k_i <= q_i, 0.0, NEG).astype(np.float32)

    in_maps = []
    for c in range(NCORES):
        wq_c = wq[:, c * QH * HD:(c + 1) * QH * HD]
        wk_c = wk[:, c * HD:(c + 1) * HD] * SCALE   # fold softmax scale into K
        wv_c = wv[:, c * HD:(c + 1) * HD]
        wqkv_c = np.ascontiguousarray(
            np.concatenate([wq_c, wk_c, wv_c], axis=1)
            .reshape(FB, 128, 768)).astype(bf16)
        wo_c = np.ascontiguousarray(
            wo[c * QH * HD:(c + 1) * QH * HD, :]
            .reshape(QH, HD, DIM)).astype(bf16)
        in_maps.append({
            "xT": xT5, "wqkv": wqkv_c, "wo4": wo_c,
            "cos4": cos4, "sin4": sin4, "diag": dmask,
        })
    return in_maps


def run_on_device(inputs, trace=False, tmpdir=None):
    """Compile (cached) + run; returns (full_output, BassKernelResults)."""
    import sys
    if "/opt/trn_rl_repo" not in sys.path:
        sys.path.insert(0, "/opt/trn_rl_repo")
    from concourse.bass_utils import run_bass_kernel_spmd

    if "nc" not in _cache:
        _cache["nc"] = _build()
    nc = _cache["nc"]
    in_maps = _prep_host(inputs)
    res = run_bass_kernel_spmd(nc, in_maps, core_ids=list(range(NCORES)),
                               trace=trace, tmpdir=tmpdir)
    acc = np.zeros((DIM // 512, TOK, 512), np.float32)
    for c in range(NCORES):
        acc += np.asarray(res.results[c]["out"], np.float32)
    full = np.ascontiguousarray(acc.transpose(1, 0, 2)).reshape(TOK, DIM)
    return full.reshape(B, S, DIM), res


def kernel(**inputs):
    out, _ = run_on_device(inputs, trace=False)
    return out
